# revision 26
# baseline (speedup 1.0000x reference)
"""Trainium2 Bass kernel for nn_BatchRNN — v3: single 16-seq chain per core,
latency-optimized LSTM step.

Each core runs ONE direction x 16 sequences in a single chain. The per-step
serial chain is minimized:
  - g-gate matmuls issue first so tanh(g) starts ~160ns into the MM group
  - sigmoid is split per gate-group (sig_if, sig_o) to overlap the MM group
  - the c update is ONE fused tensor_tensor_scan: state = f*state + p over
    interleaved [load-c, update] column pairs (p = sig_i * tanh_g)
  - stride-2 scatter views place sig_f / p / c_new exactly where the scan
    wants them, with zero extra copies
  - projection matmul groups for chunk ch+1 interleave into the scan of ch

Layouts (per core):
  xT  [D, T*S]            bf16  token-major input, col = t*S + b
  ps  [128, 128] PSUM     gates, col = m*S + b; m0,1=i m2,3=f m4,5=g m6,7=o
  Z   [128, 129]          sig_if scatter: i at even cols of [0:64],
                          f at even cols of [64:128]; d0 window = Z[63:127]
                          gives [0, f0, 0, f1, ...] (odd slots never written)
  V0/V1 [128, 65]         scan d1 double-buffer: view [1:65]; even view cols
                          = c_old (written by prev scan), odd = p (STT)
  hb  [128, (TC+1)*SL]    bf16  h history, col = (tl+1)*SL + k*S + b
"""

import sys

sys.path.insert(0, "/opt/trn_rl_repo")

import numpy as np

B, T, D, H = 64, 1024, 512, 256
H4 = 4 * H
EPS = 1e-3
P = 128
S = 16                 # sequences per core (one chain)
GROUPS = B // S        # 4
KD = D // P            # 4
KH = H // P            # 2
M8 = H4 // P           # 8
TC = 128               # time chunk
NCH = T // TC
SL = 2 * S             # 32 h cols per step (k-chunk x seq)
GW = M8 * S            # 128 gate cols per step

_COMPILED = {}
LAST_RESULT = None


def _build_graph(loop_n=None, has_bias=False):
    from concourse import bacc, bass, mybir, tile

    BF = mybir.dt.bfloat16
    F32 = mybir.dt.float32
    AF = mybir.ActivationFunctionType
    ALU = mybir.AluOpType

    nc = bacc.Bacc("TRN2", target_bir_lowering=False, debug=False, num_devices=8)

    F8 = mybir.dt.float8e4
    xT = nc.dram_tensor("xT", [D, T * S], BF, kind="ExternalInput").ap()
    wx = nc.dram_tensor("wx", [KD, P, H4], BF, kind="ExternalInput").ap()
    wh = nc.dram_tensor("wh", [KH, P, H4], F8, kind="ExternalInput").ap()
    eye = nc.dram_tensor("eye", [P, P], F8, kind="ExternalInput").ap()
    if has_bias:
        gb = nc.dram_tensor("gb", [P, M8], F32, kind="ExternalInput").ap()
    out = nc.dram_tensor("out", [P, T * SL], BF, kind="ExternalOutput").ap()

    WCH = TC * S  # 2048 token-cols per chunk

    # Gate blocks are host-permuted to [i, g, f, o]: i=m0,1 g=m2,3 f=m4,5
    # o=m6,7. i,g MMs issue first so sig_ig fires after 8 MMs (it gates
    # the critical p2->scan path); f,o after (their consumers have slack).
    MMSEQ = ([(k, m) for k in range(KH) for m in (0, 1, 2, 3)]
             + [(k, m) for k in range(KH) for m in (4, 5, 6, 7)])

    with tile.TileContext(nc) as tc:
        with (
            tc.tile_pool(name="const", bufs=1) as const,
            tc.tile_pool(name="state", bufs=1) as state,
            tc.tile_pool(name="xpool", bufs=2) as xpool,
            tc.tile_pool(name="xgpool", bufs=2) as xgpool,
            tc.tile_pool(name="hpool", bufs=2) as hpool,
            tc.tile_pool(name="psproj", bufs=2, space="PSUM") as psproj,
            tc.tile_pool(name="psscan", bufs=2, space="PSUM") as psscan,
        ):
            wx_sb = []
            for k in range(KD):
                tw = const.tile([P, H4], BF, tag=f"wx{k}")
                nc.sync.dma_start(tw[:], wx[k])
                wx_sb.append(tw)
            wh_sb = []
            for k in range(KH):
                tw = const.tile([P, H4], F8, tag=f"wh{k}")
                nc.sync.dma_start(tw[:], wh[k])
                wh_sb.append(tw)
            eye_sb = const.tile([P, P], F8, tag="eye")
            nc.sync.dma_start(eye_sb[:], eye[:])
            if has_bias:
                gbt = const.tile([P, M8], F32, tag="gbt")
                nc.sync.dma_start(gbt[:], gb[:])

            # persistent step-state tiles
            # sig_t: [si | sg] contiguous (feeds the critical p2 STT)
            # w2: sig_fo scatter: f at odd cols of [0:64] (-> scan d0 with
            # memset-zero evens), o at odd cols of [64:130]
            sig_t = state.tile([P, 2 * SL], F32, tag="sig_t", name="sig_t")
            w2 = state.tile([P, 4 * SL + 2], F32, tag="w2", name="w2")
            vt = [state.tile([P, 2 * SL + 2], F32, tag=f"v{i}", name=f"v{i}")
                  for i in range(2)]                                # scan d1 dbuf
            tcc = state.tile([P, SL], F32, tag="tcc", name="tcc")   # tanh(c)

            def dma_chunk(ch):
                xin = []
                for k in range(KD):
                    t = xpool.tile([P, WCH], BF, tag=f"xin{k}", name=f"xin{k}")
                    nc.sync.dma_start(
                        t[:], xT[k * P:(k + 1) * P, ch * WCH:(ch + 1) * WCH])
                    xin.append(t)
                return xin

            proj_state = {}

            def proj_task(xin, xg, task):
                # one proj MM (or the final copy) per scan step, so the PE
                # never blocks >~270ns on projection work
                n, m, k = task
                if k == 0:
                    proj_state[(n, m)] = psproj.tile(
                        [P, 512], F32, tag="pp", name="pp")
                ps = proj_state[(n, m)]
                if k == "c":
                    # 512 token-cols = 32 steps x 16 seqs -> xg[t, m, b]
                    xg_r = xg[:].rearrange(
                        "p (t m b) -> p t m b", t=TC, m=M8, b=S)
                    dst = xg_r[:, n * 32:(n + 1) * 32, m, :]
                    src = ps[:].rearrange("p (t b) -> p t b", b=S)
                    if has_bias:
                        nc.vector.tensor_scalar_add(dst, src, gbt[:, m:m + 1])
                    else:
                        nc.vector.tensor_copy(dst, src)
                    del proj_state[(n, m)]
                else:
                    nc.tensor.matmul(
                        ps[:],
                        wx_sb[k][:, m * P:(m + 1) * P],
                        xin[k][:, n * 512:(n + 1) * 512],
                        start=(k == 0), stop=(k == KD - 1),
                    )

            def proj_group(xin, xg, n, m):
                for k in list(range(KD)) + ["c"]:
                    proj_task(xin, xg, (n, m, k))

            def alloc_xg():
                return xgpool.tile([P, TC * GW], BF, tag="xg", name="xg")

            def body():
                nc.vector.memset(w2[:], 0.0)
                for i in range(2):
                    nc.vector.memset(vt[i][:], 0.0)
                prev_h = [None]
                xin_c = dma_chunk(0)
                xg_c = alloc_xg()
                for n in range(WCH // 512):
                    for m in range(M8):
                        proj_group(xin_c, xg_c, n, m)

                gstep = [0]  # global step counter for V parity

                for ch in range(NCH):
                    nxt = ch + 1
                    tasks = []
                    xin_n = xg_n = None
                    if nxt < NCH:
                        xin_n = dma_chunk(nxt)
                        xg_n = alloc_xg()
                        tasks = [(n, m, k) for n in range(WCH // 512)
                                 for m in range(M8)
                                 for k in list(range(KD)) + ["c"]]

                    hb = hpool.tile([P, (TC + 1) * SL], BF, tag="hb", name="hb")
                    if ch == 0:
                        nc.vector.memset(hb[:, 0:SL], 0.0)
                    else:
                        nc.vector.tensor_copy(hb[:, 0:SL], prev_h[0])

                    def step(tl):
                        va = vt[gstep[0] % 2]       # scan input (d1)
                        vb = vt[(gstep[0] + 1) % 2]  # scan output
                        gstep[0] += 1

                        ps = psscan.tile([P, GW], F32, tag="pg", name="pg")
                        # fold xg into PSUM via identity matmul
                        nc.tensor.matmul(
                            ps[:], eye_sb[:],
                            xg_c[:, tl * GW:(tl + 1) * GW],
                            start=True, stop=False, skip_group_check=True,
                        )
                        for j, (k, m) in enumerate(MMSEQ):
                            nc.tensor.matmul(
                                ps[:, m * S:(m + 1) * S],
                                wh_sb[k][:, m * P:(m + 1) * P],
                                hb[:, tl * SL + k * S: tl * SL + (k + 1) * S],
                                start=False,
                                stop=(j == len(MMSEQ) - 1),
                                skip_group_check=True,
                            )
                        # sig(i,g): critical-path ACT, fires after 8 MMs;
                        # contiguous PSUM read cols 0:64, contiguous out
                        # (g-gate weights are host-scaled x2, so sig_g = sig(2g))
                        nc.scalar.activation(sig_t[:], ps[:, 0:2 * SL],
                                             AF.Sigmoid)
                        # sig(f,o): f -> odd cols of w2[0:64] (scan d0 window
                        # w2[0:64] has memset-zero evens), o -> odd of [64:130]
                        fo_out = w2[:, 1:4 * SL + 1].rearrange(
                            "p (b c two) -> p b c two", b=2, two=2)[:, :, :, 0]
                        fo_in = ps[:, 2 * SL:4 * SL].rearrange(
                            "p (b c) -> p b c", b=2)
                        nc.scalar.activation(fo_out, fo_in, AF.Sigmoid)
                        # p/2 = (sig(2g) - 0.5) * sig_i = sig_i * tanh(g) / 2
                        # -> even cols 2..64 of va; state is c/2
                        nc.vector.scalar_tensor_tensor(
                            va[:, 2:2 * SL + 2].rearrange(
                                "p (c two) -> p c two", two=2)[:, :, 0],
                            sig_t[:, SL:2 * SL], 0.5, sig_t[:, 0:SL],
                            ALU.subtract, ALU.mult)
                        # c/2 scan: state = f*state + p/2 over [load, update]
                        # pairs; d0 = w2[0:64] = [0, f0, 0, f1, ...]
                        nc.vector.tensor_tensor_scan(
                            vb[:, 0:2 * SL], w2[:, 0:2 * SL],
                            va[:, 1:2 * SL + 1], 0.0, ALU.mult, ALU.add)
                        # tanh(c) = Tanh(2 * c/2); c_new at odd cols of vb
                        cn = vb[:, 1:2 * SL + 1].rearrange(
                            "p (c two) -> p c two", two=2)[:, :, 0]
                        nc.scalar.activation(tcc[:], cn, AF.Tanh, scale=2.0)
                        # h = sig_o * tanh_c; sig_o at odd cols of w2[64:130]
                        so_v = w2[:, 2 * SL + 1:4 * SL + 1].rearrange(
                            "p (c two) -> p c two", two=2)[:, :, 0]
                        nc.vector.tensor_mul(
                            hb[:, (tl + 1) * SL:(tl + 2) * SL], so_v, tcc[:])

                    for tl in range(TC):
                        step(tl)
                        # 160 proj tasks per 128 steps: 1/step + 1 extra
                        # every 4th step
                        npop = 1 + (tl % 4 == 3)
                        for _ in range(npop):
                            if tasks:
                                proj_task(xin_n, xg_n, tasks.pop(0))

                    nc.sync.dma_start(
                        out[:, ch * TC * SL:(ch + 1) * TC * SL], hb[:, SL:])
                    prev_h[0] = hb[:, TC * SL:(TC + 1) * SL]
                    xin_c, xg_c = xin_n, xg_n

            if loop_n is None:
                body()
            else:
                with tc.For_i(0, loop_n, 1):
                    body()

    nc.compile()
    return nc


def _get_compiled(has_bias):
    if has_bias not in _COMPILED:
        _COMPILED[has_bias] = _build_graph(has_bias=has_bias)
    return _COMPILED[has_bias]


def kernel(inputs, input_paddings, bn_scale, bn_bias, bn_mean, bn_var,
           Wx_f, Wh_f, b_f, Wx_b, Wh_b, b_b):
    from concourse import mybir
    from concourse.bass_utils import run_bass_kernel_spmd

    np_bf16 = mybir.dt.np(mybir.dt.bfloat16)
    np_f8 = mybir.dt.np(mybir.dt.float8e4)

    x = np.asarray(inputs, np.float32)
    pad = np.asarray(input_paddings, np.float32)
    keep = 1.0 - pad
    lengths = (T - pad.sum(axis=1)).astype(np.int64)
    idx = (np.arange(T - 1, -1, -1)[None, :] + lengths[:, None]) % T

    inv = ((1.0 + np.asarray(bn_scale, np.float32))
           / np.sqrt(np.asarray(bn_var, np.float32) + EPS))
    beta = np.asarray(bn_bias, np.float32) - np.asarray(bn_mean, np.float32) * inv

    x_bn = (x * inv + beta) * keep[:, :, None]
    x_flip = np.take_along_axis(x_bn, idx[:, :, None].astype(np.int64), axis=1)

    has_bias = bool(np.any(np.asarray(b_f)) or np.any(np.asarray(b_b)))

    # g-gate pre-activation scaled x2: sig(2g) = (tanh(g)+1)/2 feeds the
    # half-scaled c state on device
    gate_scale = np.ones((H4,), np.float32)
    gate_scale[2 * H:3 * H] = 2.0
    # device gate-block order [i, g, f, o] (both sigmoids read contiguous
    # PSUM ranges)
    perm = np.r_[0:H, 2 * H:3 * H, H:2 * H, 3 * H:4 * H]

    def prep_w(Wx, Wh, b):
        wxp = (np.asarray(Wx, np.float32) * gate_scale)[:, perm].astype(np_bf16)
        whp = (np.asarray(Wh, np.float32) * gate_scale)[:, perm].astype(np_f8)
        wx_t = np.stack([wxp[k * P:(k + 1) * P] for k in range(KD)])
        wh_t = np.stack([whp[k * P:(k + 1) * P] for k in range(KH)])
        gb_t = ((np.asarray(b, np.float32) * gate_scale)[perm]
                .reshape(M8, P).T.copy())
        return wx_t, wh_t, gb_t

    wx_f_t, wh_f_t, gb_f_t = prep_w(Wx_f, Wh_f, b_f)
    wx_b_t, wh_b_t, gb_b_t = prep_w(Wx_b, Wh_b, b_b)
    eye_t = np.eye(P, dtype=np.float32).astype(np_f8)

    in_maps = []
    for core in range(8):
        fwd = core < GROUPS
        g = core % GROUPS
        sl = slice(g * S, (g + 1) * S)
        xs = (x_bn if fwd else x_flip)[sl]                # [16, T, D]
        xTc = np.ascontiguousarray(xs.transpose(2, 1, 0)).reshape(D, T * S)
        im = dict(
            xT=xTc.astype(np_bf16),
            wx=(wx_f_t if fwd else wx_b_t),
            wh=(wh_f_t if fwd else wh_b_t),
            eye=eye_t,
        )
        if has_bias:
            im["gb"] = gb_f_t if fwd else gb_b_t
        in_maps.append(im)

    nc = _get_compiled(has_bias)
    res = run_bass_kernel_spmd(nc, in_maps, core_ids=list(range(8)))
    global LAST_RESULT
    LAST_RESULT = res

    out_full = np.zeros((B, T, 2 * H), np.float32)
    for core in range(8):
        fwd = core < GROUPS
        g = core % GROUPS
        sl = slice(g * S, (g + 1) * S)
        oc = np.asarray(res.results[core]["out"], dtype=np_bf16).astype(np.float32)
        # [p, t*32 + k*16 + b] -> [b, t, k*128+p]
        hs = oc.reshape(P, T, 2, S).transpose(3, 1, 2, 0).reshape(S, T, 2 * P)
        if fwd:
            out_full[sl, :, 0:H] = hs
        else:
            hs = np.take_along_axis(hs, idx[sl][:, :, None].astype(np.int64), axis=1)
            out_full[sl, :, H:2 * H] = hs
    return out_full
